# revision 36
# baseline (speedup 1.0000x reference)
"""Causal self-attention (B=4, T=2048, C=2048, H=16) on 8 trn2 NeuronCores.

Sharding: tensor-parallel over heads (2 heads/core). Each core computes the
QKV projection for its head shard (q,k produced transposed for the scores
matmul, v produced in normal layout for attn@v), applies rope fused into the
PSUM->SBUF drain, runs causal attention without max-subtraction (scores are
O(5), exp is fp32-safe), and produces yT = (attn @ v)^T per head.

All matmuls are bf16. The AllToAll that re-shards Y^T from head-column-
sharded to token-row-sharded is split into FOUR per-batch collectives, each
fired as soon as that batch's attention is done; the output projection for
batch b is emitted during batch b+1's compute so the collective latency is
hidden. Each core ends up with tokens [c*256, (c+1)*256) of every batch
(re-assembled on host). Wproj is resident in SBUF for the whole kernel.

Softmax: denominator accumulated on the PE as an all-ones [128,128] matmul
(output pre-broadcast across partitions), reciprocal on the DVE, normalize
on the DVE straight out of PSUM. All weight tensors are host-pre-transposed
to partition-major layout so their DMAs are ~128 descriptors."""

import os
import sys

os.environ.setdefault("JAX_PLATFORMS", "axon")

import numpy as np

B, T, C = 4, 2048, 2048
H = 16
HD = 128
N_CORES = 8
HL = H // N_CORES  # heads per core = 2
CL = HL * HD  # per-core head columns = 256
TQ = 512  # Tq chunk for scores
NKT = T // 128  # 16 tiles of 128 along T
KC = C // 128  # 16 k-tiles along C
TB = T // N_CORES  # tokens per (batch, core) after re-shard = 256
ROWS = B * TB  # output rows per core = 1024


def _install_ntff_shim():
    """The agent image's antenv lacks axon_hooks; provide it so
    run_bass_kernel_spmd(trace=True) can reach the NTFF profiler."""
    import types, contextlib, ctypes

    try:
        from antenv.axon_hooks import get_axon_ntff_profile_hook  # noqa

        return
    except ImportError:
        pass

    so_path = "/opt/axon/libaxon_pjrt.so"
    try:
        lib = ctypes.CDLL(so_path)
    except OSError:
        lib = None
    if lib is None or not hasattr(lib, "axon_start_nrt_profile"):
        hook = None
    else:
        lib.axon_start_nrt_profile.argtypes = [
            ctypes.POINTER(ctypes.c_int64),
            ctypes.c_size_t,
        ]
        lib.axon_start_nrt_profile.restype = ctypes.c_int64
        lib.axon_stop_nrt_profile.argtypes = [ctypes.c_char_p]
        lib.axon_stop_nrt_profile.restype = ctypes.c_int64

        @contextlib.contextmanager
        def hook(output_dir, device_ids):
            import jax

            jax.devices()
            if device_ids:
                ids = (ctypes.c_int64 * len(device_ids))(*device_ids)
                rc = lib.axon_start_nrt_profile(ids, len(device_ids))
            else:
                rc = lib.axon_start_nrt_profile(None, 0)
            if rc != 0:
                raise RuntimeError(f"axon_start_nrt_profile rc={rc}")
            try:
                yield
            finally:
                n = lib.axon_stop_nrt_profile(str(output_dir).encode())
                if n <= 0:
                    print(f"ntff profile: rc={n} (no files) dir={output_dir}")

    import antenv

    mod = types.ModuleType("antenv.axon_hooks")
    _state = {"hook": hook}
    mod.set_axon_ntff_profile_hook = lambda h: _state.__setitem__("hook", h)
    mod.get_axon_ntff_profile_hook = lambda: _state["hook"]
    sys.modules["antenv.axon_hooks"] = mod
    antenv.axon_hooks = mod


def build_program():
    import concourse.bass as bass
    import concourse.mybir as mybir
    import concourse.tile as tile
    from concourse import bacc
    from contextlib import ExitStack

    f32 = mybir.dt.float32
    f32r = mybir.dt.float32r
    bf16 = mybir.dt.bfloat16
    mdt = bf16
    Exp = mybir.ActivationFunctionType.Exp

    nc = bacc.Bacc("TRN2", target_bir_lowering=False, debug=False, num_devices=N_CORES)

    # all weights host-pre-transposed to partition-major [128, KC, *]
    xT = nc.dram_tensor("xT", [B, C, T], mdt, kind="ExternalInput")
    wqk = nc.dram_tensor("wqk", [128, KC, 4 * HD], mdt, kind="ExternalInput")
    wv = nc.dram_tensor("wv", [128, KC, CL], mdt, kind="ExternalInput")
    wproj = nc.dram_tensor("wproj", [128, KC, C], mdt, kind="ExternalInput")
    cosd = nc.dram_tensor("cos", [HD // 2, T], mdt, kind="ExternalInput")
    sind = nc.dram_tensor("sin", [HD // 2, T], mdt, kind="ExternalInput")
    out = nc.dram_tensor("out", [ROWS, C], f32, kind="ExternalOutput")

    # shard-major a2a buffers: [src/dest, p, lh, t]. Batches 0-2 re-shard in
    # one collective (256 tokens/dest); batch 3 in two half-token collectives
    # (128 tokens/dest) so the last transfer overlaps the previous proj.
    a2a_in = [
        nc.dram_tensor(f"a2a_in{b}", [N_CORES, 128, HL, TB], mdt)
        for b in range(B - 1)
    ]
    a2a_out = [
        nc.dram_tensor(f"a2a_out{b}", [N_CORES, 128, HL, TB], mdt)
        for b in range(B - 1)
    ]
    a2a3_in = [
        nc.dram_tensor(f"a2a3_in{x}", [N_CORES, 128, HL, TB // 2], mdt)
        for x in range(2)
    ]
    a2a3_out = [
        nc.dram_tensor(f"a2a3_out{x}", [N_CORES, 128, HL, TB // 2], mdt)
        for x in range(2)
    ]
    warm_in = nc.dram_tensor("warm_in", [N_CORES, 16], mdt)
    warm_out = nc.dram_tensor("warm_out", [N_CORES, 16], mdt)

    with tile.TileContext(nc) as tc:
        with ExitStack() as top:
            const = top.enter_context(tc.tile_pool(name="const", bufs=1))
            wpool = top.enter_context(tc.tile_pool(name="weights", bufs=1))

            # --- weights first on the sync queue so MMs can start early.
            # Everything is split into <=256KB pieces: a monolithic transfer
            # pins one DMA engine for tens of us and stalls the DGE config
            # stream behind it.
            wqk_sb = wpool.tile([128, KC, 4 * HD], mdt, tag="wqk")
            for ph in range(2):
                nc.sync.dma_start(
                    wqk_sb[64 * ph : 64 * (ph + 1), 0:2, :],
                    wqk[64 * ph : 64 * (ph + 1), 0:2, :],
                )
            nc.sync.dma_start(wqk_sb[:, 2:4, :], wqk[:, 2:4, :])

            # --- constants ---
            ones_f = const.tile([128, 128], f32, tag="ones_f")
            nc.vector.memset(ones_f[:], 1.0)
            ones128 = const.tile([128, 128], mdt, tag="ones128")
            nc.vector.tensor_copy(ones128[:], ones_f[:])

            cos_sb = const.tile([64, T], mdt, tag="cos")
            sin_sb = const.tile([64, T], mdt, tag="sin")
            wproj_sb = wpool.tile([128, KC, C], mdt, tag="wproj")
            wv_sb = wpool.tile([128, KC, CL], mdt, tag="wv")

            with ExitStack() as mid:
                qk_pool = mid.enter_context(tc.tile_pool(name="qkT", bufs=2))
                v_pool = mid.enter_context(tc.tile_pool(name="vsb", bufs=2))
                xk_pool = mid.enter_context(tc.tile_pool(name="xk", bufs=36))
                rtmp = mid.enter_context(tc.tile_pool(name="rtmp", bufs=1))
                apool = mid.enter_context(tc.tile_pool(name="apool", bufs=4))
                spool = mid.enter_context(tc.tile_pool(name="spool", bufs=2))
                ypool = mid.enter_context(tc.tile_pool(name="yproj", bufs=1))
                opool = mid.enter_context(tc.tile_pool(name="osb", bufs=2))
                ps_mm = mid.enter_context(
                    tc.tile_pool(name="psmm", bufs=3, space="PSUM")
                )
                ps_yT = mid.enter_context(
                    tc.tile_pool(name="psyT", bufs=2, space="PSUM")
                )
                ps_den = mid.enter_context(
                    tc.tile_pool(name="psden", bufs=2, space="PSUM")
                )
                ps_x = mid.enter_context(
                    tc.tile_pool(name="psx", bufs=1, space="PSUM")
                )

                def emit_proj(row0, ntok, y_sb, opool_=None, ps_=None):
                    opool_ = opool_ or opool
                    ps_ = ps_ or ps_mm
                    for n in range(C // TQ):
                        for m in range(ntok // 128):
                            o_ps = ps_.tile([128, TQ], f32, tag="mm")
                            for k in range(KC):
                                nc.tensor.matmul(
                                    o_ps[:],
                                    y_sb[:, k, 128 * m : 128 * (m + 1)],
                                    wproj_sb[:, k, TQ * n : TQ * (n + 1)],
                                    start=(k == 0),
                                    stop=(k == KC - 1),
                                )
                            o_sb = opool_.tile([128, TQ], f32, tag="o_sb")
                            nc.scalar.copy(o_sb[:], o_ps[:])
                            nc.sync.dma_start(
                                out[
                                    row0 + 128 * m : row0 + 128 * (m + 1),
                                    TQ * n : TQ * (n + 1),
                                ],
                                o_sb[:],
                            )

                def emit_y_load(src, tb, ypool_=None):
                    y_sb = (ypool_ or ypool).tile([128, KC, TB], mdt, tag="y")
                    for s in range(N_CORES):
                        nc.sync.dma_start(
                            y_sb[:, HL * s : HL * (s + 1), 0:tb],
                            src[s, :, :, :],
                        )
                    return y_sb

                def emit_attn_chunk(b, h, j, qkT, v_sb):
                    """One (head, query-chunk): scores/exp/mask with den and
                    yT matmuls software-pipelined one tile behind, epilogue,
                    and the a2a staging writes. The 512x512 diagonal block is
                    processed as 256-query halves so only 6 of 8 half-tiles
                    are computed (the fully-masked corners are skipped)."""
                    yT_ps = ps_yT.tile([128, TQ], f32, tag="yT")
                    den_ps = ps_den.tile([128, TQ], f32, tag="den")
                    # jobs: (i, q0, qw, first, last) — k-tile, query offset/
                    # width within the chunk, and whether this is the first/
                    # last accumulation into [q0, q0+qw)
                    HQ = TQ // 2
                    jobs = []
                    for i in range(4 * j):  # full-width tiles, no mask
                        jobs.append((i, 0, TQ, i == 0, False))
                    for qh in range(2):  # diagonal block, 256-query halves
                        nsub = 2 * qh + 2
                        for dk in range(nsub):
                            jobs.append(
                                (
                                    4 * j + dk,
                                    HQ * qh,
                                    HQ,
                                    j == 0 and dk == 0,
                                    dk == nsub - 1,
                                )
                            )
                    pend = []

                    def emit_denyT(job, a_sb):
                        i, q0, qw, first, last = job
                        nc.tensor.matmul(
                            den_ps[:, q0 : q0 + qw],
                            ones128[:],
                            a_sb[:, 0:qw],
                            start=first,
                            stop=last,
                        )
                        nc.tensor.matmul(
                            yT_ps[:, q0 : q0 + qw],
                            v_sb[:, i, 128 * h : 128 * (h + 1)],
                            a_sb[:, 0:qw],
                            start=first,
                            stop=last,
                        )

                    for job in jobs:
                        i, q0, qw, first, last = job
                        sT_ps = ps_mm.tile([128, TQ], f32, tag="mm")
                        nc.tensor.matmul(
                            sT_ps[:, 0:qw],
                            qkT[:, 2 + h, 128 * i : 128 * (i + 1)],
                            qkT[:, h, TQ * j + q0 : TQ * j + q0 + qw],
                            start=True,
                            stop=True,
                        )
                        a_sb = apool.tile([128, TQ], mdt, tag="a")
                        nc.scalar.activation(
                            a_sb[:, 0:qw], sT_ps[:, 0:qw], Exp
                        )
                        d = 128 * i - (TQ * j + q0)
                        if d > -128:
                            # causal: zero where q_local < k_global - (chunk
                            # query base), i.e. keep f >= p + d
                            nc.gpsimd.affine_select(
                                out=a_sb[:, 0:qw],
                                in_=a_sb[:, 0:qw],
                                compare_op=mybir.AluOpType.is_ge,
                                fill=0.0,
                                base=-d,
                                pattern=[[1, qw]],
                                channel_multiplier=-1,
                            )
                        if len(pend) == 2:
                            emit_denyT(*pend.pop(0))
                        pend.append((job, a_sb))
                    for p in pend:
                        emit_denyT(*p)
                    # chunk epilogue: reciprocal + normalize (the den matmul
                    # already broadcast den to all partitions)
                    rec_c = spool.tile([128, TQ], f32r, tag="rec")
                    with nc.allow_low_precision(reason="softmax recip"):
                        nc.vector.reciprocal(rec_c[:], den_ps[:])
                    yT_sb = spool.tile([128, TQ], mdt, tag="yT_sb")
                    nc.vector.tensor_mul(
                        yT_sb[:], rec_c[:].bitcast(f32), yT_ps[:]
                    )
                    if b < B - 1:
                        for u in range(2):
                            nc.sync.dma_start(
                                a2a_in[b][2 * j + u, :, h, :],
                                yT_sb[:, TB * u : TB * (u + 1)],
                            )
                    else:
                        hb = TB // 2  # 128-token shards for the split a2a
                        dst = a2a3_in[j // 2]
                        for u in range(4):
                            nc.sync.dma_start(
                                dst[4 * (j % 2) + u, :, h, :],
                                yT_sb[:, hb * u : hb * (u + 1)],
                            )

                # warm up the collectives path (ncfw/SPAD) so the first
                # real AllToAll doesn't pay cold-start while batch-1's QKV
                # is streaming
                nc.gpsimd.collective_compute(
                    "AllToAll",
                    mybir.AluOpType.bypass,
                    replica_groups=[list(range(N_CORES))],
                    ins=[warm_in[:, :]],
                    outs=[warm_out[:, :]],
                )

                def a2a(ins, outs):
                    nc.gpsimd.collective_compute(
                        "AllToAll",
                        mybir.AluOpType.bypass,
                        replica_groups=[list(range(N_CORES))],
                        ins=[ins[:, :, :, :]],
                        outs=[outs[:, :, :, :]],
                    )

                y_prev = None
                for b in range(B):
                    # ---------- QKV projection for batch b ----------
                    # qkT [128, 4, T]: m=0,1 -> qT heads 0,1 (rope+scale),
                    # m=2,3 -> kT heads 0,1 (rope). v_sb [128, NKT, CL].
                    qkT = qk_pool.tile([128, 4, T], mdt, tag="qkT")
                    v_sb = v_pool.tile([128, NKT, CL], mdt, tag="v")

                    for n in range(T // TQ):
                        xk = [
                            xk_pool.tile([128, TQ], mdt, tag="xk", name=f"xk{k}")
                            for k in range(KC)
                        ]
                        if b == 0 and n == 0:
                            # interleave the first chunk's xk tiles with the
                            # remaining weight/constant pieces, most-urgent
                            # first, all in small transfers
                            for q in range(4):
                                nc.sync.dma_start(
                                    xk[0][32 * q : 32 * (q + 1), :],
                                    xT[b, 32 * q : 32 * (q + 1), 0:TQ],
                                )
                            for k in range(1, 4):
                                for q in range(2):
                                    eng = nc.sync if q == 0 else nc.gpsimd
                                    eng.dma_start(
                                        xk[k][64 * q : 64 * (q + 1), :],
                                        xT[
                                            b,
                                            128 * k + 64 * q : 128 * k
                                            + 64 * (q + 1),
                                            0:TQ,
                                        ],
                                    )
                            for k in range(4, KC):
                                if k < 10:
                                    kw = k - 2
                                    nc.sync.dma_start(
                                        wqk_sb[:, 2 * kw : 2 * (kw + 1), :],
                                        wqk[:, 2 * kw : 2 * (kw + 1), :],
                                    )
                                eng = nc.sync if k % 2 == 0 else nc.gpsimd
                                eng.dma_start(
                                    xk[k][:],
                                    xT[b, 128 * k : 128 * (k + 1), 0:TQ],
                                )
                            nc.sync.dma_start(
                                wv_sb[:, 0:4, :], wv[:, 0:4, :]
                            )
                            nc.sync.dma_start(cos_sb[:, 0:TQ], cosd[:, 0:TQ])
                            nc.sync.dma_start(sin_sb[:, 0:TQ], sind[:, 0:TQ])
                            for kw in range(1, 4):
                                nc.sync.dma_start(
                                    wv_sb[:, 4 * kw : 4 * (kw + 1), :],
                                    wv[:, 4 * kw : 4 * (kw + 1), :],
                                )
                            for nn in range(1, 4):
                                nc.sync.dma_start(
                                    cos_sb[:, TQ * nn : TQ * (nn + 1)],
                                    cosd[:, TQ * nn : TQ * (nn + 1)],
                                )
                                nc.sync.dma_start(
                                    sin_sb[:, TQ * nn : TQ * (nn + 1)],
                                    sind[:, TQ * nn : TQ * (nn + 1)],
                                )
                        else:
                            for k in range(KC):
                                nc.sync.dma_start(
                                    xk[k][:],
                                    xT[
                                        b,
                                        128 * k : 128 * (k + 1),
                                        TQ * n : TQ * (n + 1),
                                    ],
                                )
                        for m in range(4):
                            qk_ps = ps_mm.tile([128, TQ], f32, tag="mm")
                            for k in range(KC):
                                nc.tensor.matmul(
                                    qk_ps[:],
                                    wqk_sb[:, k, 128 * m : 128 * (m + 1)],
                                    xk[k][:],
                                    start=(k == 0),
                                    stop=(k == KC - 1),
                                )
                            # rope on the PSUM->SBUF drain
                            cos_t = cos_sb[:, TQ * n : TQ * (n + 1)]
                            sin_t = sin_sb[:, TQ * n : TQ * (n + 1)]
                            t0 = rtmp.tile([64, TQ], f32, tag="t0")
                            t1 = rtmp.tile([64, TQ], f32, tag="t1")
                            nc.vector.tensor_mul(t0[:], qk_ps[0:64, :], cos_t)
                            nc.vector.tensor_mul(t1[:], qk_ps[64:128, :], sin_t)
                            nc.vector.tensor_sub(
                                qkT[0:64, m, TQ * n : TQ * (n + 1)], t0[:], t1[:]
                            )
                            t2 = rtmp.tile([64, TQ], f32, tag="t0")
                            t3 = rtmp.tile([64, TQ], f32, tag="t1")
                            nc.vector.tensor_mul(t2[:], qk_ps[64:128, :], cos_t)
                            nc.vector.tensor_mul(t3[:], qk_ps[0:64, :], sin_t)
                            nc.vector.tensor_add(
                                qkT[64:128, m, TQ * n : TQ * (n + 1)], t2[:], t3[:]
                            )
                        for m2 in range(4):
                            v_ps = ps_x.tile([128, TQ], f32, tag="x")
                            for k in range(KC):
                                nc.tensor.matmul(
                                    v_ps[:, 0:CL],
                                    xk[k][:, 128 * m2 : 128 * (m2 + 1)],
                                    wv_sb[:, k, :],
                                    start=(k == 0),
                                    stop=(k == KC - 1),
                                )
                            nc.scalar.copy(v_sb[:, 4 * n + m2, :], v_ps[:, 0:CL])

                    # ---------- attention for batch b ----------
                    if b < B - 1:
                        chunks = [(h, j) for h in range(HL) for j in range(4)]
                        for ci, (h, j) in enumerate(chunks):
                            emit_attn_chunk(b, h, j, qkT, v_sb)
                            if b < 2:
                                # pull the resident Wproj in piecewise,
                                # one 512KB piece per attention chunk of
                                # batches 0-1 (needed at proj(0), the end of
                                # batch-1 attention), on the gpsimd queue so
                                # neither the exp stream nor the xk configs
                                # are delayed and no DMA engine is pinned
                                kw = 8 * b + ci
                                nc.gpsimd.dma_start(
                                    wproj_sb[:, kw, :], wproj[:, kw, :]
                                )
                            if b > 0 and ci == 6:
                                # previous batch's a2a is complete by now;
                                # pull its Y^T in for the interleaved proj
                                # (earlier would stall the sync queue on the
                                # collective's semaphore)
                                y_prev = emit_y_load(a2a_out[b - 1], TB)
                        # proj for the previous batch overlaps this batch
                        if b > 0:
                            emit_proj(TB * (b - 1), TB, y_prev)
                        # fire this batch's re-shard once its attn is done
                        a2a(a2a_in[b], a2a_out[b])
                    else:
                        # batch 3: two half-token passes so the re-shard and
                        # projection interleave with the remaining compute
                        for h, j in [(0, 0), (1, 0), (0, 1), (1, 1)]:
                            emit_attn_chunk(b, h, j, qkT, v_sb)
                        a2a(a2a3_in[0], a2a3_out[0])
                        for ci, (h, j) in enumerate(
                            [(0, 2), (1, 2), (0, 3), (1, 3)]
                        ):
                            emit_attn_chunk(b, h, j, qkT, v_sb)
                            if ci == 0:
                                y_prev = emit_y_load(a2a_out[b - 1], TB)
                        emit_proj(TB * (b - 1), TB, y_prev)
                        a2a(a2a3_in[1], a2a3_out[1])

            # ---------- last batch's projection (two halves) ----------
            # fresh pools in the space freed by the attention stack, so the
            # y loads double-buffer and overlap the preceding projection
            with ExitStack() as tail:
                y3pool = tail.enter_context(tc.tile_pool(name="y3", bufs=2))
                opool3 = tail.enter_context(tc.tile_pool(name="osb3", bufs=3))
                ps_o3 = tail.enter_context(
                    tc.tile_pool(name="pso3", bufs=3, space="PSUM")
                )
                y3a = emit_y_load(a2a3_out[0], TB // 2, y3pool)
                emit_proj(TB * (B - 1), TB // 2, y3a, opool3, ps_o3)
                y3b = emit_y_load(a2a3_out[1], TB // 2, y3pool)
                emit_proj(TB * (B - 1) + TB // 2, TB // 2, y3b, opool3, ps_o3)

    nc.compile()
    return nc


_PERM = None


def _prep_inputs(x, rope, Wqkv, Wproj):
    """Host-side sharding/layout prep (numpy only)."""
    global _PERM
    if _PERM is None:
        _PERM = np.concatenate([np.arange(0, HD, 2), np.arange(1, HD, 2)])
    perm = _PERM

    import ml_dtypes

    mdt_np = ml_dtypes.bfloat16

    def pmajor(w):
        # [C, M] -> [128, KC, M] partition-major
        m = w.shape[1]
        return np.ascontiguousarray(
            w.reshape(KC, 128, m).transpose(1, 0, 2)
        ).astype(mdt_np)

    x = np.asarray(x, dtype=np.float32)
    xT = np.ascontiguousarray(x.transpose(0, 2, 1)).astype(mdt_np)  # [B, C, T]

    rope = np.asarray(rope, dtype=np.float32)
    cos = np.ascontiguousarray(rope[:, :, 0].T).astype(mdt_np)  # [64, T]
    sin = np.ascontiguousarray(rope[:, :, 1].T).astype(mdt_np)

    Wqkv = np.asarray(Wqkv, dtype=np.float32)
    Wq = Wqkv[:, 0:C]
    Wk = Wqkv[:, C : 2 * C]
    Wv = Wqkv[:, 2 * C : 3 * C]
    scale = 1.0 / np.sqrt(HD)
    Wproj_p = pmajor(np.asarray(Wproj, dtype=np.float32))

    in_maps = []
    for c in range(N_CORES):
        cols = []
        for lh in range(HL):
            h = HL * c + lh
            cols.append(h * HD + perm)
        qcols = np.concatenate(cols)
        wq_c = Wq[:, qcols] * scale
        wk_c = Wk[:, qcols]
        wqk_c = pmajor(np.concatenate([wq_c, wk_c], axis=1))  # [128, KC, 512]
        wv_c = pmajor(Wv[:, HL * HD * c : HL * HD * (c + 1)])  # [128, KC, 256]
        in_maps.append(
            {
                "xT": xT,
                "wqk": wqk_c,
                "wv": wv_c,
                "wproj": Wproj_p,
                "cos": cos,
                "sin": sin,
            }
        )
    return in_maps


_NC_CACHE = None


def _get_nc():
    global _NC_CACHE
    if _NC_CACHE is None:
        _NC_CACHE = build_program()
    return _NC_CACHE


def run(x, rope, Wqkv, Wproj, trace=False):
    _install_ntff_shim()
    from concourse.bass_utils import run_bass_kernel_spmd

    nc = _get_nc()
    in_maps = _prep_inputs(x, rope, Wqkv, Wproj)
    res = run_bass_kernel_spmd(nc, in_maps, list(range(N_CORES)), trace=trace)
    # batches 0-2: core c holds tokens [c*256,(c+1)*256). batch 3 (split
    # a2a): core c holds tokens [c*128,(c+1)*128) and [1024+c*128, ...+128)
    full = np.zeros((B, T, C), dtype=np.float32)
    hb = TB // 2
    for c in range(N_CORES):
        o = res.results[c]["out"].reshape(B, TB, C)
        full[: B - 1, c * TB : (c + 1) * TB, :] = o[: B - 1]
        full[B - 1, c * hb : (c + 1) * hb, :] = o[B - 1, 0:hb]
        full[B - 1, T // 2 + c * hb : T // 2 + (c + 1) * hb, :] = o[B - 1, hb:TB]
    return full, res


def kernel(x, rope, Wqkv, Wproj):
    out, _ = run(x, rope, Wqkv, Wproj, trace=False)
    return out


if __name__ == "__main__":
    import time

    t0 = time.time()
    nc = build_program()
    ni = sum(len(bb.instructions) for f in nc.m.functions for bb in f.blocks)
    print(f"build ok: {time.time()-t0:.1f}s, {ni} instructions")


# revision 37
# speedup vs baseline: 1.0129x; 1.0129x over previous
"""Causal self-attention (B=4, T=2048, C=2048, H=16) on 8 trn2 NeuronCores.

Sharding: tensor-parallel over heads (2 heads/core). Each core computes the
QKV projection for its head shard (q,k produced transposed for the scores
matmul, v produced in normal layout for attn@v), applies rope fused into the
PSUM->SBUF drain, runs causal attention without max-subtraction (scores are
O(5), exp is fp32-safe), and produces yT = (attn @ v)^T per head.

All matmuls are bf16. The AllToAll that re-shards Y^T from head-column-
sharded to token-row-sharded is split into FOUR per-batch collectives, each
fired as soon as that batch's attention is done; the output projection for
batch b is emitted during batch b+1's compute so the collective latency is
hidden. Each core ends up with tokens [c*256, (c+1)*256) of every batch
(re-assembled on host). Wproj is resident in SBUF for the whole kernel.

Softmax: denominator accumulated on the PE as an all-ones [128,128] matmul
(output pre-broadcast across partitions), reciprocal on the DVE, normalize
on the DVE straight out of PSUM. All weight tensors are host-pre-transposed
to partition-major layout so their DMAs are ~128 descriptors."""

import os
import sys

os.environ.setdefault("JAX_PLATFORMS", "axon")

import numpy as np

B, T, C = 4, 2048, 2048
H = 16
HD = 128
N_CORES = 8
HL = H // N_CORES  # heads per core = 2
CL = HL * HD  # per-core head columns = 256
TQ = 512  # Tq chunk for scores
NKT = T // 128  # 16 tiles of 128 along T
KC = C // 128  # 16 k-tiles along C
TB = T // N_CORES  # tokens per (batch, core) after re-shard = 256
ROWS = B * TB  # output rows per core = 1024


def _install_ntff_shim():
    """The agent image's antenv lacks axon_hooks; provide it so
    run_bass_kernel_spmd(trace=True) can reach the NTFF profiler."""
    import types, contextlib, ctypes

    try:
        from antenv.axon_hooks import get_axon_ntff_profile_hook  # noqa

        return
    except ImportError:
        pass

    so_path = "/opt/axon/libaxon_pjrt.so"
    try:
        lib = ctypes.CDLL(so_path)
    except OSError:
        lib = None
    if lib is None or not hasattr(lib, "axon_start_nrt_profile"):
        hook = None
    else:
        lib.axon_start_nrt_profile.argtypes = [
            ctypes.POINTER(ctypes.c_int64),
            ctypes.c_size_t,
        ]
        lib.axon_start_nrt_profile.restype = ctypes.c_int64
        lib.axon_stop_nrt_profile.argtypes = [ctypes.c_char_p]
        lib.axon_stop_nrt_profile.restype = ctypes.c_int64

        @contextlib.contextmanager
        def hook(output_dir, device_ids):
            import jax

            jax.devices()
            if device_ids:
                ids = (ctypes.c_int64 * len(device_ids))(*device_ids)
                rc = lib.axon_start_nrt_profile(ids, len(device_ids))
            else:
                rc = lib.axon_start_nrt_profile(None, 0)
            if rc != 0:
                raise RuntimeError(f"axon_start_nrt_profile rc={rc}")
            try:
                yield
            finally:
                n = lib.axon_stop_nrt_profile(str(output_dir).encode())
                if n <= 0:
                    print(f"ntff profile: rc={n} (no files) dir={output_dir}")

    import antenv

    mod = types.ModuleType("antenv.axon_hooks")
    _state = {"hook": hook}
    mod.set_axon_ntff_profile_hook = lambda h: _state.__setitem__("hook", h)
    mod.get_axon_ntff_profile_hook = lambda: _state["hook"]
    sys.modules["antenv.axon_hooks"] = mod
    antenv.axon_hooks = mod


def build_program():
    import concourse.bass as bass
    import concourse.mybir as mybir
    import concourse.tile as tile
    from concourse import bacc
    from contextlib import ExitStack

    f32 = mybir.dt.float32
    f32r = mybir.dt.float32r
    bf16 = mybir.dt.bfloat16
    mdt = bf16
    Exp = mybir.ActivationFunctionType.Exp

    nc = bacc.Bacc("TRN2", target_bir_lowering=False, debug=False, num_devices=N_CORES)

    # all weights host-pre-transposed to partition-major [128, KC, *]
    xT = nc.dram_tensor("xT", [B, C, T], mdt, kind="ExternalInput")
    wqk = nc.dram_tensor("wqk", [128, KC, 4 * HD], mdt, kind="ExternalInput")
    wv = nc.dram_tensor("wv", [128, KC, CL], mdt, kind="ExternalInput")
    wproj = nc.dram_tensor("wproj", [128, KC, C], mdt, kind="ExternalInput")
    cosd = nc.dram_tensor("cos", [HD // 2, T], mdt, kind="ExternalInput")
    sind = nc.dram_tensor("sin", [HD // 2, T], mdt, kind="ExternalInput")
    out = nc.dram_tensor("out", [ROWS, C], f32, kind="ExternalOutput")

    # shard-major a2a buffers: [src/dest, p, lh, t]. Batches 0-2 re-shard in
    # one collective (256 tokens/dest); batch 3 in two half-token collectives
    # (128 tokens/dest) so the last transfer overlaps the previous proj.
    a2a_in = [
        nc.dram_tensor(f"a2a_in{b}", [N_CORES, 128, HL, TB], mdt)
        for b in range(B - 1)
    ]
    a2a_out = [
        nc.dram_tensor(f"a2a_out{b}", [N_CORES, 128, HL, TB], mdt)
        for b in range(B - 1)
    ]
    a2a3_in = [
        nc.dram_tensor(f"a2a3_in{x}", [N_CORES, 128, HL, TB // 2], mdt)
        for x in range(2)
    ]
    a2a3_out = [
        nc.dram_tensor(f"a2a3_out{x}", [N_CORES, 128, HL, TB // 2], mdt)
        for x in range(2)
    ]
    warm_in = nc.dram_tensor("warm_in", [N_CORES, 16], mdt)
    warm_out = nc.dram_tensor("warm_out", [N_CORES, 16], mdt)

    with tile.TileContext(nc) as tc:
        with ExitStack() as top:
            const = top.enter_context(tc.tile_pool(name="const", bufs=1))
            wpool = top.enter_context(tc.tile_pool(name="weights", bufs=1))

            # --- weights first on the sync queue so MMs can start early.
            # Everything is split into <=256KB pieces: a monolithic transfer
            # pins one DMA engine for tens of us and stalls the DGE config
            # stream behind it.
            wqk_sb = wpool.tile([128, KC, 4 * HD], mdt, tag="wqk")
            for ph in range(2):
                nc.sync.dma_start(
                    wqk_sb[64 * ph : 64 * (ph + 1), 0:2, :],
                    wqk[64 * ph : 64 * (ph + 1), 0:2, :],
                )
            nc.sync.dma_start(wqk_sb[:, 2:4, :], wqk[:, 2:4, :])

            # --- constants ---
            ones_f = const.tile([128, 128], f32, tag="ones_f")
            nc.vector.memset(ones_f[:], 1.0)
            ones128 = const.tile([128, 128], mdt, tag="ones128")
            nc.vector.tensor_copy(ones128[:], ones_f[:])

            cos_sb = const.tile([64, T], mdt, tag="cos")
            sin_sb = const.tile([64, T], mdt, tag="sin")
            wproj_sb = wpool.tile([128, KC, C], mdt, tag="wproj")
            wv_sb = wpool.tile([128, KC, CL], mdt, tag="wv")

            with ExitStack() as mid:
                qk_pool = mid.enter_context(tc.tile_pool(name="qkT", bufs=2))
                v_pool = mid.enter_context(tc.tile_pool(name="vsb", bufs=2))
                xk_pool = mid.enter_context(tc.tile_pool(name="xk", bufs=36))
                rtmp = mid.enter_context(tc.tile_pool(name="rtmp", bufs=1))
                apool = mid.enter_context(tc.tile_pool(name="apool", bufs=4))
                spool = mid.enter_context(tc.tile_pool(name="spool", bufs=2))
                ypool = mid.enter_context(tc.tile_pool(name="yproj", bufs=1))
                opool = mid.enter_context(tc.tile_pool(name="osb", bufs=2))
                ps_mm = mid.enter_context(
                    tc.tile_pool(name="psmm", bufs=3, space="PSUM")
                )
                ps_yT = mid.enter_context(
                    tc.tile_pool(name="psyT", bufs=2, space="PSUM")
                )
                ps_den = mid.enter_context(
                    tc.tile_pool(name="psden", bufs=2, space="PSUM")
                )
                ps_x = mid.enter_context(
                    tc.tile_pool(name="psx", bufs=1, space="PSUM")
                )

                def emit_proj(row0, ntok, y_sb, opool_=None, ps_=None):
                    opool_ = opool_ or opool
                    ps_ = ps_ or ps_mm
                    for n in range(C // TQ):
                        for m in range(ntok // 128):
                            o_ps = ps_.tile([128, TQ], f32, tag="mm")
                            for k in range(KC):
                                nc.tensor.matmul(
                                    o_ps[:],
                                    y_sb[:, k, 128 * m : 128 * (m + 1)],
                                    wproj_sb[:, k, TQ * n : TQ * (n + 1)],
                                    start=(k == 0),
                                    stop=(k == KC - 1),
                                )
                            o_sb = opool_.tile([128, TQ], f32, tag="o_sb")
                            nc.scalar.copy(o_sb[:], o_ps[:])
                            nc.sync.dma_start(
                                out[
                                    row0 + 128 * m : row0 + 128 * (m + 1),
                                    TQ * n : TQ * (n + 1),
                                ],
                                o_sb[:],
                            )

                def emit_y_load(src, tb, ypool_=None):
                    y_sb = (ypool_ or ypool).tile([128, KC, TB], mdt, tag="y")
                    for s in range(N_CORES):
                        nc.sync.dma_start(
                            y_sb[:, HL * s : HL * (s + 1), 0:tb],
                            src[s, :, :, :],
                        )
                    return y_sb

                def emit_attn_chunk(b, h, j, qkT, v_sb):
                    """One (head, query-chunk): scores/exp/mask with den and
                    yT matmuls software-pipelined one tile behind, epilogue,
                    and the a2a staging writes. The 512x512 diagonal block is
                    processed as 256-query halves so only 6 of 8 half-tiles
                    are computed (the fully-masked corners are skipped)."""
                    yT_ps = ps_yT.tile([128, TQ], f32, tag="yT")
                    den_ps = ps_den.tile([128, TQ], f32, tag="den")
                    # jobs: (i, q0, qw, first, last) — k-tile, query offset/
                    # width within the chunk, and whether this is the first/
                    # last accumulation into [q0, q0+qw)
                    HQ = TQ // 2
                    jobs = []
                    for i in range(4 * j):  # full-width tiles, no mask
                        jobs.append((i, 0, TQ, i == 0, False))
                    for qh in range(2):  # diagonal block, 256-query halves
                        nsub = 2 * qh + 2
                        for dk in range(nsub):
                            jobs.append(
                                (
                                    4 * j + dk,
                                    HQ * qh,
                                    HQ,
                                    j == 0 and dk == 0,
                                    dk == nsub - 1,
                                )
                            )
                    pend = []

                    def emit_denyT(job, a_sb):
                        i, q0, qw, first, last = job
                        nc.tensor.matmul(
                            den_ps[:, q0 : q0 + qw],
                            ones128[:],
                            a_sb[:, 0:qw],
                            start=first,
                            stop=last,
                        )
                        nc.tensor.matmul(
                            yT_ps[:, q0 : q0 + qw],
                            v_sb[:, i, 128 * h : 128 * (h + 1)],
                            a_sb[:, 0:qw],
                            start=first,
                            stop=last,
                        )

                    for job in jobs:
                        i, q0, qw, first, last = job
                        sT_ps = ps_mm.tile([128, TQ], f32, tag="mm")
                        nc.tensor.matmul(
                            sT_ps[:, 0:qw],
                            qkT[:, 2 + h, 128 * i : 128 * (i + 1)],
                            qkT[:, h, TQ * j + q0 : TQ * j + q0 + qw],
                            start=True,
                            stop=True,
                        )
                        a_sb = apool.tile([128, TQ], mdt, tag="a")
                        nc.scalar.activation(
                            a_sb[:, 0:qw], sT_ps[:, 0:qw], Exp
                        )
                        d = 128 * i - (TQ * j + q0)
                        if d > -128:
                            # causal: zero where q_local < k_global - (chunk
                            # query base), i.e. keep f >= p + d
                            nc.gpsimd.affine_select(
                                out=a_sb[:, 0:qw],
                                in_=a_sb[:, 0:qw],
                                compare_op=mybir.AluOpType.is_ge,
                                fill=0.0,
                                base=-d,
                                pattern=[[1, qw]],
                                channel_multiplier=-1,
                            )
                        if len(pend) == 2:
                            emit_denyT(*pend.pop(0))
                        pend.append((job, a_sb))
                    for p in pend:
                        emit_denyT(*p)
                    # chunk epilogue: reciprocal + normalize (the den matmul
                    # already broadcast den to all partitions)
                    rec_c = spool.tile([128, TQ], f32r, tag="rec")
                    with nc.allow_low_precision(reason="softmax recip"):
                        nc.vector.reciprocal(rec_c[:], den_ps[:])
                    yT_sb = spool.tile([128, TQ], mdt, tag="yT_sb")
                    nc.vector.tensor_mul(
                        yT_sb[:], rec_c[:].bitcast(f32), yT_ps[:]
                    )
                    if b < B - 1:
                        for u in range(2):
                            nc.sync.dma_start(
                                a2a_in[b][2 * j + u, :, h, :],
                                yT_sb[:, TB * u : TB * (u + 1)],
                            )
                    else:
                        hb = TB // 2  # 128-token shards for the split a2a
                        dst = a2a3_in[j // 2]
                        for u in range(4):
                            nc.sync.dma_start(
                                dst[4 * (j % 2) + u, :, h, :],
                                yT_sb[:, hb * u : hb * (u + 1)],
                            )

                # warm up the collectives path (ncfw/SPAD) so the first
                # real AllToAll doesn't pay cold-start while batch-1's QKV
                # is streaming
                nc.gpsimd.collective_compute(
                    "AllToAll",
                    mybir.AluOpType.bypass,
                    replica_groups=[list(range(N_CORES))],
                    ins=[warm_in[:, :]],
                    outs=[warm_out[:, :]],
                )

                def a2a(ins, outs):
                    nc.gpsimd.collective_compute(
                        "AllToAll",
                        mybir.AluOpType.bypass,
                        replica_groups=[list(range(N_CORES))],
                        ins=[ins[:, :, :, :]],
                        outs=[outs[:, :, :, :]],
                    )

                y_prev = None
                for b in range(B):
                    # ---------- QKV projection for batch b ----------
                    # qkT [128, 4, T]: m=0,1 -> qT heads 0,1 (rope+scale),
                    # m=2,3 -> kT heads 0,1 (rope). v_sb [128, NKT, CL].
                    qkT = qk_pool.tile([128, 4, T], mdt, tag="qkT")
                    v_sb = v_pool.tile([128, NKT, CL], mdt, tag="v")

                    for n in range(T // TQ):
                        xk = [
                            xk_pool.tile([128, TQ], mdt, tag="xk", name=f"xk{k}")
                            for k in range(KC)
                        ]
                        if b == 0 and n == 0:
                            # interleave the first chunk's xk tiles with the
                            # remaining weight/constant pieces, most-urgent
                            # first, all in small transfers
                            for q in range(4):
                                nc.sync.dma_start(
                                    xk[0][32 * q : 32 * (q + 1), :],
                                    xT[b, 32 * q : 32 * (q + 1), 0:TQ],
                                )
                            for k in range(1, 4):
                                for q in range(2):
                                    nc.sync.dma_start(
                                        xk[k][64 * q : 64 * (q + 1), :],
                                        xT[
                                            b,
                                            128 * k + 64 * q : 128 * k
                                            + 64 * (q + 1),
                                            0:TQ,
                                        ],
                                    )
                            for k in range(4, KC):
                                if k < 10:
                                    kw = k - 2
                                    nc.sync.dma_start(
                                        wqk_sb[:, 2 * kw : 2 * (kw + 1), :],
                                        wqk[:, 2 * kw : 2 * (kw + 1), :],
                                    )
                                nc.sync.dma_start(
                                    xk[k][:],
                                    xT[b, 128 * k : 128 * (k + 1), 0:TQ],
                                )
                            nc.sync.dma_start(
                                wv_sb[:, 0:4, :], wv[:, 0:4, :]
                            )
                            nc.sync.dma_start(cos_sb[:, 0:TQ], cosd[:, 0:TQ])
                            nc.sync.dma_start(sin_sb[:, 0:TQ], sind[:, 0:TQ])
                            for kw in range(1, 4):
                                nc.sync.dma_start(
                                    wv_sb[:, 4 * kw : 4 * (kw + 1), :],
                                    wv[:, 4 * kw : 4 * (kw + 1), :],
                                )
                            for nn in range(1, 4):
                                nc.sync.dma_start(
                                    cos_sb[:, TQ * nn : TQ * (nn + 1)],
                                    cosd[:, TQ * nn : TQ * (nn + 1)],
                                )
                                nc.sync.dma_start(
                                    sin_sb[:, TQ * nn : TQ * (nn + 1)],
                                    sind[:, TQ * nn : TQ * (nn + 1)],
                                )
                        else:
                            for k in range(KC):
                                nc.sync.dma_start(
                                    xk[k][:],
                                    xT[
                                        b,
                                        128 * k : 128 * (k + 1),
                                        TQ * n : TQ * (n + 1),
                                    ],
                                )
                        for m in range(4):
                            qk_ps = ps_mm.tile([128, TQ], f32, tag="mm")
                            for k in range(KC):
                                nc.tensor.matmul(
                                    qk_ps[:],
                                    wqk_sb[:, k, 128 * m : 128 * (m + 1)],
                                    xk[k][:],
                                    start=(k == 0),
                                    stop=(k == KC - 1),
                                )
                            # rope on the PSUM->SBUF drain
                            cos_t = cos_sb[:, TQ * n : TQ * (n + 1)]
                            sin_t = sin_sb[:, TQ * n : TQ * (n + 1)]
                            t0 = rtmp.tile([64, TQ], f32, tag="t0")
                            t1 = rtmp.tile([64, TQ], f32, tag="t1")
                            nc.vector.tensor_mul(t0[:], qk_ps[0:64, :], cos_t)
                            nc.vector.tensor_mul(t1[:], qk_ps[64:128, :], sin_t)
                            nc.vector.tensor_sub(
                                qkT[0:64, m, TQ * n : TQ * (n + 1)], t0[:], t1[:]
                            )
                            t2 = rtmp.tile([64, TQ], f32, tag="t0")
                            t3 = rtmp.tile([64, TQ], f32, tag="t1")
                            nc.vector.tensor_mul(t2[:], qk_ps[64:128, :], cos_t)
                            nc.vector.tensor_mul(t3[:], qk_ps[0:64, :], sin_t)
                            nc.vector.tensor_add(
                                qkT[64:128, m, TQ * n : TQ * (n + 1)], t2[:], t3[:]
                            )
                        for m2 in range(4):
                            v_ps = ps_x.tile([128, TQ], f32, tag="x")
                            for k in range(KC):
                                nc.tensor.matmul(
                                    v_ps[:, 0:CL],
                                    xk[k][:, 128 * m2 : 128 * (m2 + 1)],
                                    wv_sb[:, k, :],
                                    start=(k == 0),
                                    stop=(k == KC - 1),
                                )
                            nc.scalar.copy(v_sb[:, 4 * n + m2, :], v_ps[:, 0:CL])

                    # ---------- attention for batch b ----------
                    if b < B - 1:
                        chunks = [(h, j) for h in range(HL) for j in range(4)]
                        for ci, (h, j) in enumerate(chunks):
                            emit_attn_chunk(b, h, j, qkT, v_sb)
                            if b < 2:
                                # pull the resident Wproj in piecewise,
                                # one 512KB piece per attention chunk of
                                # batches 0-1 (needed at proj(0), the end of
                                # batch-1 attention), on the gpsimd queue so
                                # neither the exp stream nor the xk configs
                                # are delayed and no DMA engine is pinned
                                kw = 8 * b + ci
                                nc.gpsimd.dma_start(
                                    wproj_sb[:, kw, :], wproj[:, kw, :]
                                )
                            if b > 0 and ci == 6:
                                # previous batch's a2a is complete by now;
                                # pull its Y^T in for the interleaved proj
                                # (earlier would stall the sync queue on the
                                # collective's semaphore)
                                y_prev = emit_y_load(a2a_out[b - 1], TB)
                        # proj for the previous batch overlaps this batch
                        if b > 0:
                            emit_proj(TB * (b - 1), TB, y_prev)
                        # fire this batch's re-shard once its attn is done
                        a2a(a2a_in[b], a2a_out[b])
                    else:
                        # batch 3: two half-token passes so the re-shard and
                        # projection interleave with the remaining compute
                        for h, j in [(0, 0), (1, 0), (0, 1), (1, 1)]:
                            emit_attn_chunk(b, h, j, qkT, v_sb)
                        a2a(a2a3_in[0], a2a3_out[0])
                        for ci, (h, j) in enumerate(
                            [(0, 2), (1, 2), (0, 3), (1, 3)]
                        ):
                            emit_attn_chunk(b, h, j, qkT, v_sb)
                            if ci == 0:
                                y_prev = emit_y_load(a2a_out[b - 1], TB)
                        emit_proj(TB * (b - 1), TB, y_prev)
                        a2a(a2a3_in[1], a2a3_out[1])

            # ---------- last batch's projection (two halves) ----------
            # fresh pools in the space freed by the attention stack, so the
            # y loads double-buffer and overlap the preceding projection
            with ExitStack() as tail:
                y3pool = tail.enter_context(tc.tile_pool(name="y3", bufs=2))
                opool3 = tail.enter_context(tc.tile_pool(name="osb3", bufs=3))
                ps_o3 = tail.enter_context(
                    tc.tile_pool(name="pso3", bufs=3, space="PSUM")
                )
                y3a = emit_y_load(a2a3_out[0], TB // 2, y3pool)
                emit_proj(TB * (B - 1), TB // 2, y3a, opool3, ps_o3)
                y3b = emit_y_load(a2a3_out[1], TB // 2, y3pool)
                emit_proj(TB * (B - 1) + TB // 2, TB // 2, y3b, opool3, ps_o3)

    nc.compile()
    return nc


_PERM = None


def _prep_inputs(x, rope, Wqkv, Wproj):
    """Host-side sharding/layout prep (numpy only)."""
    global _PERM
    if _PERM is None:
        _PERM = np.concatenate([np.arange(0, HD, 2), np.arange(1, HD, 2)])
    perm = _PERM

    import ml_dtypes

    mdt_np = ml_dtypes.bfloat16

    def pmajor(w):
        # [C, M] -> [128, KC, M] partition-major
        m = w.shape[1]
        return np.ascontiguousarray(
            w.reshape(KC, 128, m).transpose(1, 0, 2)
        ).astype(mdt_np)

    x = np.asarray(x, dtype=np.float32)
    xT = np.ascontiguousarray(x.transpose(0, 2, 1)).astype(mdt_np)  # [B, C, T]

    rope = np.asarray(rope, dtype=np.float32)
    cos = np.ascontiguousarray(rope[:, :, 0].T).astype(mdt_np)  # [64, T]
    sin = np.ascontiguousarray(rope[:, :, 1].T).astype(mdt_np)

    Wqkv = np.asarray(Wqkv, dtype=np.float32)
    Wq = Wqkv[:, 0:C]
    Wk = Wqkv[:, C : 2 * C]
    Wv = Wqkv[:, 2 * C : 3 * C]
    scale = 1.0 / np.sqrt(HD)
    Wproj_p = pmajor(np.asarray(Wproj, dtype=np.float32))

    in_maps = []
    for c in range(N_CORES):
        cols = []
        for lh in range(HL):
            h = HL * c + lh
            cols.append(h * HD + perm)
        qcols = np.concatenate(cols)
        wq_c = Wq[:, qcols] * scale
        wk_c = Wk[:, qcols]
        wqk_c = pmajor(np.concatenate([wq_c, wk_c], axis=1))  # [128, KC, 512]
        wv_c = pmajor(Wv[:, HL * HD * c : HL * HD * (c + 1)])  # [128, KC, 256]
        in_maps.append(
            {
                "xT": xT,
                "wqk": wqk_c,
                "wv": wv_c,
                "wproj": Wproj_p,
                "cos": cos,
                "sin": sin,
            }
        )
    return in_maps


_NC_CACHE = None


def _get_nc():
    global _NC_CACHE
    if _NC_CACHE is None:
        _NC_CACHE = build_program()
    return _NC_CACHE


def run(x, rope, Wqkv, Wproj, trace=False):
    _install_ntff_shim()
    from concourse.bass_utils import run_bass_kernel_spmd

    nc = _get_nc()
    in_maps = _prep_inputs(x, rope, Wqkv, Wproj)
    res = run_bass_kernel_spmd(nc, in_maps, list(range(N_CORES)), trace=trace)
    # batches 0-2: core c holds tokens [c*256,(c+1)*256). batch 3 (split
    # a2a): core c holds tokens [c*128,(c+1)*128) and [1024+c*128, ...+128)
    full = np.zeros((B, T, C), dtype=np.float32)
    hb = TB // 2
    for c in range(N_CORES):
        o = res.results[c]["out"].reshape(B, TB, C)
        full[: B - 1, c * TB : (c + 1) * TB, :] = o[: B - 1]
        full[B - 1, c * hb : (c + 1) * hb, :] = o[B - 1, 0:hb]
        full[B - 1, T // 2 + c * hb : T // 2 + (c + 1) * hb, :] = o[B - 1, hb:TB]
    return full, res


def kernel(x, rope, Wqkv, Wproj):
    out, _ = run(x, rope, Wqkv, Wproj, trace=False)
    return out


if __name__ == "__main__":
    import time

    t0 = time.time()
    nc = build_program()
    ni = sum(len(bb.instructions) for f in nc.m.functions for bb in f.blocks)
    print(f"build ok: {time.time()-t0:.1f}s, {ni} instructions")
